# revision 1
# baseline (speedup 1.0000x reference)
"""Multi-head attention (B=4, S=2048, D=1024, H=16) on 8 trn2 NeuronCores.

Sharding: batch x head-half. Core c handles batch b = c//2 and heads
hh*8..hh*8+8 where hh = c%2. Each core computes its heads' Q/K/V
projections, attention, and a partial output projection; the host sums
the two partials per batch and adds the (constant) bias terms.

Device-side layout choices (per core, S=2048, DH=512 head dims):
  q_dT, k_dT : [depth-dims, S]  (f32, matmul-ready: contraction on partitions)
  scores_T   : [Sk, Sq] tiles  = k_dT.T @ q_dT            (f32r matmuls, K=64,
               head pairs packed into PE row-groups 0-1 / 2-3)
  attn       : exp on ACT (PSUM->SBUF, bf16), multiplicative binary mask on DVE
  PV         : o_T[depth, Sq] += v[Sk,depth].T-style matmul, bf16, with a
               parallel M=1 ones-matmul producing softmax row-sums
  normalize  : reciprocal + K=1 broadcast matmul + DVE multiply
  out proj   : out[Sq, 1024] = o_T.T @ WoT  (f32r), DMA to DRAM

Scale 1/sqrt(depth) is folded into Wq/bq on the host. bv and bo are folded
into a constant host-side bias (attention rows sum to 1).
"""

import numpy as np

D = 1024
S = 2048
HPC = 8          # heads per core
DH = HPC * 64    # 512 per-core head dims
N_CORES = 8

_CACHE = {}


def _build_program(reps=1):
    if reps in _CACHE:
        return _CACHE[reps]

    from concourse import bacc, tile, mybir

    f32 = mybir.dt.float32
    f32r = mybir.dt.float32r
    bf16 = mybir.dt.bfloat16
    AF = mybir.ActivationFunctionType

    nc = bacc.Bacc(
        "TRN2",
        target_bir_lowering=False,
        debug=False,
        enable_asserts=False,
        num_devices=N_CORES,
    )

    xqT = nc.dram_tensor("xqT", [D, S], f32r, kind="ExternalInput").ap()
    xkT = nc.dram_tensor("xkT", [D, S], f32r, kind="ExternalInput").ap()
    xvT = nc.dram_tensor("xvT", [D, S], f32r, kind="ExternalInput").ap()
    wqT = nc.dram_tensor("wqT", [D, DH], f32r, kind="ExternalInput").ap()
    wkT = nc.dram_tensor("wkT", [D, DH], f32r, kind="ExternalInput").ap()
    wvT = nc.dram_tensor("wvT", [D, DH], f32r, kind="ExternalInput").ap()
    woT = nc.dram_tensor("woT", [DH, D], bf16, kind="ExternalInput").ap()
    bq_d = nc.dram_tensor("bq", [DH], f32, kind="ExternalInput").ap()
    bk_d = nc.dram_tensor("bk", [DH], f32, kind="ExternalInput").ap()
    mmul = nc.dram_tensor("maskmul", [S, S], bf16, kind="ExternalInput").ap()
    out = nc.dram_tensor("out", [S, D], f32, kind="ExternalOutput").ap()

    with tile.TileContext(nc) as tc:
        with (
            nc.allow_low_precision(reason="f32r is fp32-width; rounding intended"),
            tc.tile_pool(name="big", bufs=1) as big,
            tc.tile_pool(name="ot", bufs=2) as otp,
            tc.tile_pool(name="wp", bufs=2) as wp,
            tc.tile_pool(name="stream", bufs=3) as stream,
            tc.tile_pool(name="small", bufs=3) as small,
            tc.tile_pool(name="aux", bufs=2) as auxp,
            tc.tile_pool(name="ps", bufs=2, space="PSUM") as psp,
            tc.tile_pool(name="po", bufs=4, space="PSUM") as pop,
        ):
            # ---- persistent tiles ----
            q_dT = big.tile([128, 4, S], f32r, tag="q_dT")
            k_dT = big.tile([128, 4, S], f32r, tag="k_dT")
            v_sb = big.tile([128, 16, HPC, 65], bf16, tag="v_sb")
            woT_sb = big.tile([128, 4, D], bf16, tag="woT_sb")
            bias_q = big.tile([128, 4], f32, tag="bias_q")
            bias_k = big.tile([128, 4], f32, tag="bias_k")
            nc.vector.memset(v_sb[:, :, :, 64:65], 1.0)
            nc.sync.dma_start(bias_q[:], bq_d.rearrange("(c p) -> p c", p=128))
            nc.sync.dma_start(bias_k[:], bk_d.rearrange("(c p) -> p c", p=128))
            woT_r = woT.rearrange("(c p) n -> p c n", p=128)
            for c4 in range(4):
                nc.sync.dma_start(woT_sb[:, c4], woT_r[:, c4])

            for _rep in range(reps):
                # ---- K projection:  k_dT[d, s] = (Wk xk^T)[d, s] + bk[d] ----
                wk_sb = wp.tile([128, 8, DH], f32r, tag="w", name="wk_sb")
                wr = wkT.rearrange("(kc p) m -> p kc m", p=128)
                for kc in range(8):
                    nc.sync.dma_start(wk_sb[:, kc], wr[:, kc])
                xr = xkT.rearrange("(kc p) s -> p kc s", p=128)
                for st in range(4):
                    xt = stream.tile([128, 8, 512], f32r, tag="stream")
                    for kc in range(8):
                        nc.sync.dma_start(xt[:, kc], xr[:, kc, st * 512 : (st + 1) * 512])
                    for mc in range(4):
                        ps = psp.tile([128, 1024], f32, tag="ps")
                        for kc in range(8):
                            nc.tensor.matmul(
                                ps[:, :512],
                                wk_sb[:, kc, mc * 128 : (mc + 1) * 128],
                                xt[:, kc],
                                start=(kc == 0),
                                stop=(kc == 7),
                            )
                        # per-partition bias add fused into the PSUM->SBUF
                        # copy on ScalarE (keeps DVE free for the mask path)
                        nc.scalar.activation(
                            k_dT[:, mc, st * 512 : (st + 1) * 512],
                            ps[:, :512],
                            AF.Identity,
                            bias=bias_k[:, mc : mc + 1],
                        )

                # ---- Q projection, one Sq-tile at a time. st=0 runs up front;
                # st>0 is emitted inside the previous tile's attention loop so
                # the scheduler can drop its matmuls into PE idle gaps (ACT is
                # the pacer during attention).
                wq_sb = wp.tile([128, 8, DH], f32r, tag="w", name="wq_sb")
                wqr = wqT.rearrange("(kc p) m -> p kc m", p=128)
                for kc in range(8):
                    nc.sync.dma_start(wq_sb[:, kc], wqr[:, kc])
                xq_r = xqT.rearrange("(kc p) s -> p kc s", p=128)
                qx_tiles = {}
                qp_ps = {}

                def emit_qproj(st, mc, half):
                    # half-size filler unit (~0.85us of PE) so a burst never
                    # exceeds the exp backlog ACT has banked up
                    if mc == 0 and half == 0:
                        qx = stream.tile(
                            [128, 8, 512], f32r, tag="stream", name=f"qx{st}"
                        )
                        for kc in range(8):
                            nc.sync.dma_start(
                                qx[:, kc], xq_r[:, kc, st * 512 : (st + 1) * 512]
                            )
                        qx_tiles[st] = qx
                    qx = qx_tiles[st]
                    if half == 0:
                        qp_ps[(st, mc)] = pop.tile(
                            [128, 512], f32, tag="T", name=f"qp{st}_{mc}"
                        )
                    ps = qp_ps[(st, mc)]
                    for kc in range(half * 4, half * 4 + 4):
                        nc.tensor.matmul(
                            ps[:],
                            wq_sb[:, kc, mc * 128 : (mc + 1) * 128],
                            qx[:, kc],
                            start=(kc == 0),
                            stop=(kc == 7),
                        )
                    if half == 1:
                        nc.scalar.activation(
                            q_dT[:, mc, st * 512 : (st + 1) * 512],
                            ps[:],
                            AF.Identity,
                            bias=bias_q[:, mc : mc + 1],
                        )

                # ---- V projection: v[sk, dv] (bf16, per-head layout). Emitted
                # as per-sk-chunk groups inside the FIRST attention head-pair's
                # loop (right before the PV matmul that consumes that chunk), so
                # ACT starts the exp stream while V is still being projected.
                wv_sb = wp.tile([128, 8, DH], f32r, tag="w", name="wv_sb")
                wvr = wvT.rearrange("(kc p) m -> p kc m", p=128)
                for kc in range(8):
                    nc.sync.dma_start(wv_sb[:, kc], wvr[:, kc])
                xv_r = xvT.rearrange("(kc p) s -> p kc s", p=128)
                vx_tiles = {}

                def emit_vproj(skc):
                    sg, s4 = divmod(skc, 4)
                    if s4 == 0:
                        vx = stream.tile(
                            [128, 8, 512], f32r, tag="stream", name=f"vx{sg}"
                        )
                        for kc in range(8):
                            nc.sync.dma_start(
                                vx[:, kc], xv_r[:, kc, sg * 512 : (sg + 1) * 512]
                            )
                        vx_tiles[sg] = vx
                    vx = vx_tiles[sg]
                    ps = pop.tile([128, 512], f32, tag="T", name=f"vp{skc}")
                    for kc in range(8):
                        nc.tensor.matmul(
                            ps[:],
                            vx[:, kc, s4 * 128 : (s4 + 1) * 128],
                            wv_sb[:, kc],
                            start=(kc == 0),
                            stop=(kc == 7),
                        )
                    nc.vector.tensor_copy(
                        v_sb[:, skc, :, :64],
                        ps[:].rearrange("p (h d) -> p h d", h=HPC),
                    )

                # ---- output projection for one (st, nh, q4) block; emitted as
                # PE filler inside the NEXT Sq-tile's attention loop ----
                def emit_c_group(st_c, oT_tile, nh, q4):
                    pc = pop.tile([128, 512], f32, tag="T", name=f"pc{st_c}_{nh}_{q4}")
                    for c4 in range(4):
                        nc.tensor.matmul(
                            pc[:],
                            oT_tile[:, c4, q4 * 128 : (q4 + 1) * 128],
                            woT_sb[:, c4, nh * 512 : (nh + 1) * 512],
                            start=(c4 == 0),
                            stop=(c4 == 3),
                        )
                    ob = auxp.tile([128, 512], f32, tag="ob", name=f"ob{st_c}_{nh}_{q4}")
                    nc.vector.tensor_copy(ob[:], pc[:])
                    nc.sync.dma_start(
                        out[
                            st_c * 512 + q4 * 128 : st_c * 512 + (q4 + 1) * 128,
                            nh * 512 : (nh + 1) * 512,
                        ],
                        ob[:],
                    )

                for mc in range(4):
                    emit_qproj(0, mc, 0)
                    emit_qproj(0, mc, 1)

                # ---- attention per 512-wide Sq tile, with next-tile q-proj and
                # previous-tile output-proj interleaved as PE filler ----
                mq = mmul.rearrange("(c p) s -> p c s", p=128)  # [128, 16, S]
                prev = None
                filler = []
                for st in range(4):
                    sq = slice(st * 512, (st + 1) * 512)
                    msk = stream.tile([128, 16, 512], bf16, tag="stream")
                    for c4 in range(4):
                        nc.sync.dma_start(
                            msk[:, c4 * 4 : (c4 + 1) * 4],
                            mq[:, c4 * 4 : (c4 + 1) * 4, sq],
                        )
                    oT_sb = otp.tile([128, 4, 512], bf16, tag="oT_sb")
                    for hp in range(4):
                        Ts = [pop.tile([65, 512], f32, tag="T", name=f"T{i}") for i in range(2)]
                        for skc in range(16):
                            # deferred PE filler: emit mid-loop so the next
                            # head-pair's first scores aren't delayed behind it
                            if skc in (4, 8, 12) and filler:
                                filler.pop(0)()
                            if st == 0 and hp == 0:
                                emit_vproj(skc)
                            sk = slice(skc * 128, (skc + 1) * 128)
                            ps_s = psp.tile([128, 1024], f32, tag="ps")
                            for par in range(2):
                                b0 = par * 64
                                nc.tensor.matmul(
                                    ps_s[:, par * 512 : (par + 1) * 512],
                                    k_dT[b0 : b0 + 64, hp, sk],
                                    q_dT[b0 : b0 + 64, hp, sq],
                                    start=True,
                                    stop=True,
                                )
                            ex = small.tile([128, 1024], bf16, tag="ex")
                            nc.scalar.activation(ex[:], ps_s[:], AF.Exp)
                            # binary mask applied multiplicatively, in place
                            nc.vector.tensor_mul(
                                ex.rearrange("p (t s) -> p t s", t=2),
                                ex.rearrange("p (t s) -> p t s", t=2),
                                msk[:, skc, None, :].to_broadcast((128, 2, 512)),
                            )
                            for par in range(2):
                                # v_aug column 64 is all-ones: psum row 64
                                # accumulates the masked softmax denominator
                                nc.tensor.matmul(
                                    Ts[par][:],
                                    v_sb[:, skc, hp * 2 + par],
                                    ex[:, par * 512 : (par + 1) * 512],
                                    start=(skc == 0),
                                    stop=(skc == 15),
                                )
                        for par in range(2):
                            rc = auxp.tile([128, 512], f32, tag="rc")
                            nc.vector.reciprocal(rc[64:65], Ts[par][64:65])
                            # broadcast the reciprocal row across 64 partitions via
                            # DMA (free-dim step-0 source + partition relocation);
                            # PE/DVE cannot broadcast or shift across partitions
                            bcs = auxp.tile([128, 512], f32, tag="bcs")
                            nc.sync.dma_start(
                                bcs[0:64],
                                rc[64:65, None, :].to_broadcast((1, 64, 512)),
                            )
                            if par == 0:
                                nc.vector.tensor_mul(
                                    oT_sb[0:64, hp, :], Ts[par][0:64], bcs[0:64]
                                )
                            else:
                                # par1's head dims live at partitions 64-127 of
                                # oT_sb; DVE cannot shift partitions, DMA can
                                on = auxp.tile([64, 512], bf16, tag="on")
                                nc.vector.tensor_mul(on[:], Ts[par][0:64], bcs[0:64])
                                nc.sync.dma_start(oT_sb[64:128, hp, :], on[:])
                        # queue PE filler work; it is emitted mid-skc-loop of the
                        # following head pair (ACT-paced steady state has ~20%
                        # PE idle to absorb it)
                        if st < 3:
                            filler.append(
                                lambda st=st, hp=hp: (
                                    emit_qproj(st + 1, hp, 0),
                                    emit_qproj(st + 1, hp, 1),
                                )
                            )
                        if prev is not None:
                            for j in (2 * hp, 2 * hp + 1):
                                filler.append(
                                    lambda p=prev, j=j: emit_c_group(
                                        p[0], p[1], j // 4, j % 4
                                    )
                                )
                    prev = (st, oT_sb)
                while filler:
                    filler.pop(0)()
                for j in range(8):
                    emit_c_group(3, prev[1], j // 4, j % 4)

    nc.compile()
    _CACHE[reps] = nc
    return nc


def _prepare_in_maps(q_in, k_in, v_in, m_in, Wq, bq, Wk, bk, Wv, Wo):
    import ml_dtypes

    bf16 = ml_dtypes.bfloat16
    f32 = np.float32

    per_half = []
    for hh in range(2):
        sl = slice(hh * DH, (hh + 1) * DH)
        per_half.append(
            dict(
                wqT=np.ascontiguousarray((Wq[sl, :] / 8.0).T, f32),
                wkT=np.ascontiguousarray(Wk[sl, :].T, f32),
                wvT=np.ascontiguousarray(Wv[sl, :].T, f32),
                woT=np.ascontiguousarray(Wo[:, sl].T, f32).astype(bf16),
                bq=np.ascontiguousarray(bq[sl] / 8.0, f32),
                bk=np.ascontiguousarray(bk[sl], f32),
            )
        )

    in_maps = []
    for b in range(4):
        xqT = np.ascontiguousarray(q_in[b].T, f32)
        xkT = np.ascontiguousarray(k_in[b].T, f32)
        xvT = np.ascontiguousarray(v_in[b].T, f32)
        maskmul = np.ascontiguousarray((1.0 - m_in[b, 0].T)).astype(bf16)
        for hh in range(2):
            m = dict(xqT=xqT, xkT=xkT, xvT=xvT, maskmul=maskmul)
            m.update(per_half[hh])
            in_maps.append(m)
    return in_maps


def _run(inputs, trace=False, trace_kwargs=None):
    from concourse import bass_utils

    q_in = np.asarray(inputs["q_in"], np.float32)
    k_in = np.asarray(inputs["k_in"], np.float32)
    v_in = np.asarray(inputs["v_in"], np.float32)
    m_in = np.asarray(inputs["m_in"], np.float32)
    Wq = np.asarray(inputs["Wq"], np.float32)
    bq = np.asarray(inputs["bq"], np.float32)
    Wk = np.asarray(inputs["Wk"], np.float32)
    bk = np.asarray(inputs["bk"], np.float32)
    Wv = np.asarray(inputs["Wv"], np.float32)
    bv = np.asarray(inputs["bv"], np.float32)
    Wo = np.asarray(inputs["Wo"], np.float32)
    bo = np.asarray(inputs["bo"], np.float32)

    nc = _build_program()
    in_maps = _prepare_in_maps(q_in, k_in, v_in, m_in, Wq, bq, Wk, bk, Wv, Wo)
    kw = {}
    if trace:
        kw["trace"] = True
        if trace_kwargs:
            kw["trace_kwargs"] = trace_kwargs
    res = bass_utils.run_bass_kernel_spmd(
        nc, in_maps, core_ids=list(range(N_CORES)), **kw
    )

    total_bias = (bo + bv @ Wo.T).astype(np.float32)
    output = np.empty((4, S, D), np.float32)
    for b in range(4):
        output[b] = res.results[2 * b]["out"] + res.results[2 * b + 1]["out"]
        output[b] += total_bias
    return output, res


def kernel(**inputs) -> np.ndarray:
    output, _ = _run(inputs, trace=False)
    return output


def run_traced(inputs):
    """For test.py: returns (output, BassKernelResults with exec_time_ns)."""
    return _run(inputs, trace=True)

